# revision 15
# baseline (speedup 1.0000x reference)
"""Trainium2 Bass kernel for nn_Conv2d_24833500905755.

Computes the reference's "mismatched flatten order" conv:
  out[b,co,h,w] = sum_{c,di,dj} xpad[b,c,h+di,w+dj] * Wt[c, di*3+dj, co]
with Wt = K.reshape(576, C_OUT).reshape(C_IN, 9, C_OUT).

Strategy (data-parallel over 8 cores, 4 images per core):
  - Host: scramble K into Wt, shard x on batch, replicate Wt.
  - Core: pack 2 images on the 128-partition dim (C_IN=64 each half).
    DMA each image pair into a zero-padded [128, 58, 58] SBUF tile.
    For each 8-row output chunk, accumulate 9 shifted matmuls per
    image half into PSUM (K=64 contraction in partition rows 0-63 /
    64-127 -> concurrent PE row-group tiles), copy PSUM -> SBUF,
    one big DMA per image back to HBM.
"""

import numpy as np

import concourse.bass as bass
import concourse.mybir as mybir
from concourse.bass_utils import run_bass_kernel_spmd
from concourse.tile import TileContext
from concourse.vector_clock import ScopedClock


_WAIT_LIMIT = 1


class PatchedTileContext(TileContext):
    """The container's walrus rejects instructions carrying more than one
    semaphore wait ("Too many sync wait commands"). Hoist excess waits onto
    same-engine NoOps committed just before, and split the kernel-tail Drain
    into a chain of single-wait drains."""

    def _commit_instruction(self, inst, lazy_reg_writes=True):
        si = getattr(inst, "sync_info", None)
        if (
            si is not None
            and si.on_wait is not None
            and len(si.on_wait) > _WAIT_LIMIT
            and inst.engine != mybir.EngineType.Unassigned
        ):
            waits = list(si.on_wait)
            extra, keep = waits[:-_WAIT_LIMIT], waits[-_WAIT_LIMIT:]
            for i in range(0, len(extra), _WAIT_LIMIT):
                noop = mybir.InstNoOp(
                    name=f"{inst.name}_hw{i}",
                    engine=inst.engine,
                    sync_info=mybir.SyncInfo(
                        on_wait=extra[i : i + _WAIT_LIMIT], on_update=[]
                    ),
                    bass_nofuse=True,
                )
                super()._commit_instruction(noop, lazy_reg_writes=False)
            inst.sync_info.on_wait = keep
        return super()._commit_instruction(inst, lazy_reg_writes=lazy_reg_writes)

    def _drain_and_barrier(self, tick_clock, wait_clock):
        nc = self.nc
        drain_inst = nc.sync.drain()
        wait_clock.add_sem_waits(
            drain_inst.ins, ScopedClock({None: tick_clock.global_clock})
        )
        waits = list(drain_inst.ins.sync_info.on_wait)
        if len(waits) > 1:
            drain_inst.ins.sync_info.on_wait = [waits[0]]
            num2handle = {h.num: h for h in self.sems.allocated().values()}
            for w in waits[1:]:
                d2 = nc.sync.drain()
                d2.wait_op(num2handle[w.id], w.wait_value, "sem-ge")
        nc.all_engine_barrier()
        assert self.sems is not None
        popped = nc._tile_sem_poison_stack.pop()
        assert popped is self._sem_poison
        nc.clear_and_free_semaphores(list(self.sems.allocated().values()))
        nc.all_engine_barrier()

B, C_IN, C_OUT, H = 32, 64, 128, 56
KS = 3
N_CORES = 8
BPC = B // N_CORES        # images per core
HP = H + 2               # padded height/width (pad=1)
RCHUNK = 8               # output rows per PSUM tile (8*56=448 <= 512 fp32/bank)
NCHUNK = H // RCHUNK     # 7

# matmul input dtype: float32 (safe) or float32r (4x faster, ~1.4e-4 rel err)
MM_DT = mybir.dt.float32r


def build_nc(mm_dt=MM_DT):
    f32 = mybir.dt.float32
    nc = bass.Bass()
    # x arrives pre-padded (1-px zero border) from the host
    x_ext = nc.declare_dram_parameter("x", [BPC, C_IN, HP, HP], mm_dt, isOutput=False)
    w_ext = nc.declare_dram_parameter("w", [2 * C_IN, KS * KS, C_OUT], mm_dt, isOutput=False)
    out_ext = nc.declare_dram_parameter("out", [BPC, C_OUT, H, H], f32, isOutput=True)

    with PatchedTileContext(nc) as tc:
        with (
            tc.tile_pool(name="wp", bufs=1) as wpool,
            tc.tile_pool(name="xp", bufs=2) as xpool,
            tc.tile_pool(name="op", bufs=2) as opool,
            tc.tile_pool(name="ps", bufs=4, space="PSUM") as pspool,
        ):
            wt = wpool.tile([2 * C_IN, KS * KS, C_OUT], mm_dt)
            # gpsimd ring: dispatches before Sync finishes its preamble
            nc.gpsimd.dma_start(out=wt[:], in_=w_ext[:])

            # output staging blocks (DMA'd out as soon as filled): rows
            # [0,24) after chunk 2, [24,56) after chunk 6
            OBLOCKS = [(0, 24), (24, 40), (40, 56)]
            for p in range(BPC // 2):  # image pairs
                xp = xpool.tile([2 * C_IN, HP, HP], mm_dt)
                src = x_ext[2 * p : 2 * p + 2].rearrange("b c h w -> (b c) h w")
                # split the load so early chunks unblock sooner
                eng0 = nc.gpsimd if p == 0 else nc.sync
                eng0.dma_start(out=xp[:, 0:12, :], in_=src[:, 0:12, :])
                nc.sync.dma_start(out=xp[:, 12:34, :], in_=src[:, 12:34, :])
                nc.sync.dma_start(out=xp[:, 34:HP, :], in_=src[:, 34:HP, :])

                for ci in range(NCHUNK):
                    h0 = ci * RCHUNK
                    blo, bhi = next(b for b in OBLOCKS if b[0] <= h0 < b[1])
                    if h0 == blo:
                        obs = [
                            opool.tile([C_OUT, bhi - blo, H], f32, tag=f"ob{i}",
                                       name=f"ob{i}_{p}_{h0}")
                            for i in range(2)
                        ]
                    pss = [pspool.tile([C_OUT, RCHUNK, H], f32, tag=f"ps{j}", name=f"ps{j}_{p}_{ci}") for j in range(2)]
                    for k in range(KS * KS):
                        di, dj = divmod(k, KS)
                        for half in range(2):
                            c0 = half * C_IN
                            nc.tensor.matmul(
                                out=pss[half][:],
                                lhsT=wt[c0 : c0 + C_IN, k, :],
                                rhs=xp[c0 : c0 + C_IN, h0 + di : h0 + di + RCHUNK, dj : dj + H],
                                start=(k == 0),
                                stop=(k == KS * KS - 1),
                            )
                    for half in range(2):
                        nc.any.tensor_copy(
                            out=obs[half][:, h0 - blo : h0 - blo + RCHUNK, :],
                            in_=pss[half][:],
                        )
                    if h0 + RCHUNK == bhi:
                        for half in range(2):
                            dst = out_ext[2 * p + half : 2 * p + half + 1].rearrange(
                                "b c h w -> (b c) h w"
                            )
                            # scalar-engine HWDGE ring: keeps Sync free for loads
                            nc.scalar.dma_start(
                                out=dst[:, blo:bhi, :], in_=obs[half][:]
                            )
    return nc


def _prep_inputs(x, K):
    x = np.ascontiguousarray(np.asarray(x, dtype=np.float32))
    K = np.ascontiguousarray(np.asarray(K, dtype=np.float32))
    xpad = np.pad(x, ((0, 0), (0, 0), (1, 1), (1, 1)))
    Wt = K.reshape(KS * KS * C_IN, C_OUT).reshape(C_IN, KS * KS, C_OUT)
    Wrep = np.ascontiguousarray(np.concatenate([Wt, Wt], axis=0))  # [128, 9, C_OUT]
    shards = xpad.reshape(N_CORES, BPC, C_IN, HP, HP)
    return [{"x": np.ascontiguousarray(shards[i]), "w": Wrep} for i in range(N_CORES)]


def run(x, K, trace=False, mm_dt=MM_DT):
    nc = build_nc(mm_dt)
    in_maps = _prep_inputs(x, K)
    res = run_bass_kernel_spmd(nc, in_maps, list(range(N_CORES)), trace=trace)
    out = np.concatenate([res.results[i]["out"] for i in range(N_CORES)], axis=0)
    return out, res


def kernel(x, K):
    out, _ = run(x, K, trace=False)
    return out


# revision 16
# speedup vs baseline: 1.1071x; 1.1071x over previous
"""Trainium2 Bass kernel for nn_Conv2d_24833500905755.

Computes the reference's "mismatched flatten order" conv:
  out[b,co,h,w] = sum_{c,di,dj} xpad[b,c,h+di,w+dj] * Wt[c, di*3+dj, co]
with Wt = K.reshape(576, C_OUT).reshape(C_IN, 9, C_OUT).

Strategy (data-parallel over 8 cores, 4 images per core):
  - Host: scramble K into Wt, shard x on batch, replicate Wt.
  - Core: pack 2 images on the 128-partition dim (C_IN=64 each half).
    DMA each image pair into a zero-padded [128, 58, 58] SBUF tile.
    For each 8-row output chunk, accumulate 9 shifted matmuls per
    image half into PSUM (K=64 contraction in partition rows 0-63 /
    64-127 -> concurrent PE row-group tiles), copy PSUM -> SBUF,
    one big DMA per image back to HBM.
"""

import numpy as np

import concourse.bass as bass
import concourse.mybir as mybir
from concourse.bass_utils import run_bass_kernel_spmd
from concourse.tile import TileContext
from concourse.vector_clock import ScopedClock


_WAIT_LIMIT = 1


class PatchedTileContext(TileContext):
    """The container's walrus rejects instructions carrying more than one
    semaphore wait ("Too many sync wait commands"). Hoist excess waits onto
    same-engine NoOps committed just before, and split the kernel-tail Drain
    into a chain of single-wait drains."""

    def _commit_instruction(self, inst, lazy_reg_writes=True):
        si = getattr(inst, "sync_info", None)
        if (
            si is not None
            and si.on_wait is not None
            and len(si.on_wait) > _WAIT_LIMIT
            and inst.engine != mybir.EngineType.Unassigned
        ):
            waits = list(si.on_wait)
            extra, keep = waits[:-_WAIT_LIMIT], waits[-_WAIT_LIMIT:]
            for i in range(0, len(extra), _WAIT_LIMIT):
                noop = mybir.InstNoOp(
                    name=f"{inst.name}_hw{i}",
                    engine=inst.engine,
                    sync_info=mybir.SyncInfo(
                        on_wait=extra[i : i + _WAIT_LIMIT], on_update=[]
                    ),
                    bass_nofuse=True,
                )
                super()._commit_instruction(noop, lazy_reg_writes=False)
            inst.sync_info.on_wait = keep
        return super()._commit_instruction(inst, lazy_reg_writes=lazy_reg_writes)

    def _drain_and_barrier(self, tick_clock, wait_clock):
        nc = self.nc
        drain_inst = nc.sync.drain()
        wait_clock.add_sem_waits(
            drain_inst.ins, ScopedClock({None: tick_clock.global_clock})
        )
        waits = list(drain_inst.ins.sync_info.on_wait)
        if len(waits) > 1:
            drain_inst.ins.sync_info.on_wait = [waits[0]]
            num2handle = {h.num: h for h in self.sems.allocated().values()}
            for w in waits[1:]:
                d2 = nc.sync.drain()
                d2.wait_op(num2handle[w.id], w.wait_value, "sem-ge")
        nc.all_engine_barrier()
        assert self.sems is not None
        popped = nc._tile_sem_poison_stack.pop()
        assert popped is self._sem_poison
        nc.clear_and_free_semaphores(list(self.sems.allocated().values()))
        nc.all_engine_barrier()

B, C_IN, C_OUT, H = 32, 64, 128, 56
KS = 3
N_CORES = 8
BPC = B // N_CORES        # images per core
HP = H + 2               # padded height/width (pad=1)
RCHUNK = 8               # output rows per PSUM tile (8*56=448 <= 512 fp32/bank)
NCHUNK = H // RCHUNK     # 7

# matmul input dtype: float32 (safe) or float32r (4x faster, ~1.4e-4 rel err)
MM_DT = mybir.dt.float32r


def build_nc(mm_dt=MM_DT):
    f32 = mybir.dt.float32
    nc = bass.Bass()
    # x arrives pre-padded (1-px zero border) from the host
    x_ext = nc.declare_dram_parameter("x", [BPC, C_IN, HP, HP], mm_dt, isOutput=False)
    w_ext = nc.declare_dram_parameter("w", [2 * C_IN, KS * KS, C_OUT], mm_dt, isOutput=False)
    out_ext = nc.declare_dram_parameter("out", [BPC, C_OUT, H, H], f32, isOutput=True)

    with PatchedTileContext(nc) as tc:
        with (
            tc.tile_pool(name="wp", bufs=1) as wpool,
            tc.tile_pool(name="xp", bufs=2) as xpool,
            tc.tile_pool(name="op", bufs=2) as opool,
            tc.tile_pool(name="ps", bufs=4, space="PSUM") as pspool,
        ):
            wt = wpool.tile([2 * C_IN, KS * KS, C_OUT], mm_dt)
            nc.sync.dma_start(out=wt[:], in_=w_ext[:])

            # output staging blocks (DMA'd out as soon as filled): rows
            # [0,24) after chunk 2, [24,56) after chunk 6
            OBLOCKS = [(0, 24), (24, 40), (40, 56)]
            for p in range(BPC // 2):  # image pairs
                xp = xpool.tile([2 * C_IN, HP, HP], mm_dt)
                src = x_ext[2 * p : 2 * p + 2].rearrange("b c h w -> (b c) h w")
                # split the load so early chunks unblock sooner
                nc.sync.dma_start(out=xp[:, 0:12, :], in_=src[:, 0:12, :])
                nc.sync.dma_start(out=xp[:, 12:34, :], in_=src[:, 12:34, :])
                nc.sync.dma_start(out=xp[:, 34:HP, :], in_=src[:, 34:HP, :])

                for ci in range(NCHUNK):
                    h0 = ci * RCHUNK
                    blo, bhi = next(b for b in OBLOCKS if b[0] <= h0 < b[1])
                    if h0 == blo:
                        obs = [
                            opool.tile([C_OUT, bhi - blo, H], f32, tag=f"ob{i}",
                                       name=f"ob{i}_{p}_{h0}")
                            for i in range(2)
                        ]
                    pss = [pspool.tile([C_OUT, RCHUNK, H], f32, tag=f"ps{j}", name=f"ps{j}_{p}_{ci}") for j in range(2)]
                    for k in range(KS * KS):
                        di, dj = divmod(k, KS)
                        for half in range(2):
                            c0 = half * C_IN
                            nc.tensor.matmul(
                                out=pss[half][:],
                                lhsT=wt[c0 : c0 + C_IN, k, :],
                                rhs=xp[c0 : c0 + C_IN, h0 + di : h0 + di + RCHUNK, dj : dj + H],
                                start=(k == 0),
                                stop=(k == KS * KS - 1),
                            )
                    for half in range(2):
                        nc.any.tensor_copy(
                            out=obs[half][:, h0 - blo : h0 - blo + RCHUNK, :],
                            in_=pss[half][:],
                        )
                    if h0 + RCHUNK == bhi:
                        for half in range(2):
                            dst = out_ext[2 * p + half : 2 * p + half + 1].rearrange(
                                "b c h w -> (b c) h w"
                            )
                            # scalar-engine HWDGE ring: keeps Sync free for loads
                            nc.scalar.dma_start(
                                out=dst[:, blo:bhi, :], in_=obs[half][:]
                            )
    return nc


def _prep_inputs(x, K):
    x = np.ascontiguousarray(np.asarray(x, dtype=np.float32))
    K = np.ascontiguousarray(np.asarray(K, dtype=np.float32))
    xpad = np.pad(x, ((0, 0), (0, 0), (1, 1), (1, 1)))
    Wt = K.reshape(KS * KS * C_IN, C_OUT).reshape(C_IN, KS * KS, C_OUT)
    Wrep = np.ascontiguousarray(np.concatenate([Wt, Wt], axis=0))  # [128, 9, C_OUT]
    shards = xpad.reshape(N_CORES, BPC, C_IN, HP, HP)
    return [{"x": np.ascontiguousarray(shards[i]), "w": Wrep} for i in range(N_CORES)]


def run(x, K, trace=False, mm_dt=MM_DT):
    nc = build_nc(mm_dt)
    in_maps = _prep_inputs(x, K)
    res = run_bass_kernel_spmd(nc, in_maps, list(range(N_CORES)), trace=trace)
    out = np.concatenate([res.results[i]["out"] for i in range(N_CORES)], axis=0)
    return out, res


def kernel(x, K):
    out, _ = run(x, K, trace=False)
    return out


# revision 17
# speedup vs baseline: 1.2837x; 1.1595x over previous
"""Trainium2 Bass kernel for nn_Conv2d_24833500905755 (3x3 conv, B=32,
C_in=64, C_out=128, 56x56, pad 1, with the reference's mismatched
weight-flatten order).

Math: out[b,co,h,w] = sum_{c,di,dj} xpad[b,c,h+di,w+dj] * Wt[c,di*3+dj,co]
with Wt = K.reshape(576, C_OUT).reshape(C_IN, 9, C_OUT).

Data-parallel: 4 images per NeuronCore, 2 images packed on the
128-partition dim (fp16 matmuls, K=64 contraction per half, concurrent
PE row-group tiles). Raw-bass hand-scheduled engine programs:

Same math as kernel.py, but hand-scheduled engine programs with manual
semaphores to avoid the Tile preamble/cleanup overhead (~12us):
  Sync:   w + pair-0 input DMAs (3 pieces), final output-completion wait
  Scalar: pair-1 input DMAs, then per-block output DMAs
  Tensor: 252 fp32r matmuls, gated on input-piece / bank-WAR sems
  Vector: 28 PSUM->SBUF copies
"""

from contextlib import ExitStack

import numpy as np

import concourse.bass as bass
import concourse.mybir as mybir
from concourse.bass_utils import run_bass_kernel_spmd

B, C_IN, C_OUT, H = 32, 64, 128, 56
KS = 3
N_CORES = 8
BPC = B // N_CORES
HP = H + 2
RCHUNK = 8
NCHUNK = H // RCHUNK          # 7 chunks/image, 14 global chunks (2 pairs)
OBLOCKS = [(0, 24), (24, 40), (40, 48), (48, 56)]
MM_DT = mybir.dt.float16


def build_nc(mm_dt=MM_DT):
    f32 = mybir.dt.float32
    nc = bass.Bass()
    x_ext = nc.declare_dram_parameter("x", [BPC, C_IN, HP, HP], mm_dt, isOutput=False)
    w_ext = nc.declare_dram_parameter("w", [2 * C_IN, KS * KS, C_OUT], mm_dt, isOutput=False)
    out_ext = nc.declare_dram_parameter("out", [BPC, C_OUT, H, H], f32, isOutput=True)

    n_out_dmas = 2 * len(OBLOCKS) * 2  # pairs * blocks * halves

    with ExitStack() as ctx:
        wt = ctx.enter_context(nc.sbuf_tensor("wt", [2 * C_IN, KS * KS, C_OUT], mm_dt))
        xps = [
            ctx.enter_context(nc.sbuf_tensor(f"xp{p}", [2 * C_IN, HP, HP], mm_dt))
            for p in range(2)
        ]
        # obs[p][half][block]
        obs = [
            [
                [
                    ctx.enter_context(
                        nc.sbuf_tensor(f"ob_{p}_{h}_{bi}", [C_OUT, bhi - blo, H], f32)
                    )
                    for bi, (blo, bhi) in enumerate(OBLOCKS)
                ]
                for h in range(2)
            ]
            for p in range(2)
        ]
        # banks[slot][half] - 8 PSUM banks
        banks = [
            [
                ctx.enter_context(
                    nc.psum_tensor(f"ps_{s}_{h}", [C_OUT, RCHUNK, H], f32)
                )
                for h in range(2)
            ]
            for s in range(4)
        ]
        s_w = ctx.enter_context(nc.semaphore("s_w"))
        s_x = [ctx.enter_context(nc.semaphore(f"s_x{p}")) for p in range(2)]
        s_xa = ctx.enter_context(nc.semaphore("s_xa"))
        s_mm = ctx.enter_context(nc.semaphore("s_mm"))
        s_cp = ctx.enter_context(nc.semaphore("s_cp"))
        s_cp2 = ctx.enter_context(nc.semaphore("s_cp2"))
        s_out = ctx.enter_context(nc.semaphore("s_out"))

        with nc.Block() as block:

            @block.sync
            def _(sync: bass.BassEngine):
                src = x_ext[0:2].rearrange("b c h w -> (b c) h w")
                sync.dma_start(out=xps[0][:, 0:11, :], in_=src[:, 0:11, :]).then_inc(s_xa, 16)
                sync.dma_start(out=xps[0][:, 11:34, :], in_=src[:, 11:34, :]).then_inc(s_x[0], 16)
                sync.dma_start(out=xps[0][:, 34:HP, :], in_=src[:, 34:HP, :]).then_inc(s_x[0], 16)
                for p in range(2):
                    for bi, (blo, bhi) in enumerate(OBLOCKS):
                        c_last = p * NCHUNK + (bhi // RCHUNK - 1)
                        sync.wait_ge(s_cp, c_last + 1)
                        dst = out_ext[2 * p : 2 * p + 1].rearrange("b c h w -> (b c) h w")
                        sync.dma_start(
                            out=dst[:, blo:bhi, :], in_=obs[p][0][bi][:]
                        ).then_inc(s_out, 16)
                sync.wait_ge(s_out, 16 * n_out_dmas)

            @block.scalar
            def _(scalar: bass.BassEngine):
                scalar.dma_start(out=wt[:], in_=w_ext[:]).then_inc(s_w, 16)
                src = x_ext[2:4].rearrange("b c h w -> (b c) h w")
                scalar.dma_start(out=xps[1][:, 0:12, :], in_=src[:, 0:12, :]).then_inc(s_x[1], 16)
                scalar.dma_start(out=xps[1][:, 12:34, :], in_=src[:, 12:34, :]).then_inc(s_x[1], 16)
                scalar.dma_start(out=xps[1][:, 34:HP, :], in_=src[:, 34:HP, :]).then_inc(s_x[1], 16)
                for p in range(2):
                    for ci in range(NCHUNK):
                        c = p * NCHUNK + ci
                        h0 = ci * RCHUNK
                        blo, bhi = next(b for b in OBLOCKS if b[0] <= h0 < b[1])
                        bi = OBLOCKS.index((blo, bhi))
                        scalar.wait_ge(s_mm, 2 * (c + 1))
                        scalar.copy(
                            out=obs[p][1][bi][:, h0 - blo : h0 - blo + RCHUNK, :],
                            in_=banks[c % 4][1][:],
                        ).then_inc(s_cp2, 1)
                        if h0 + RCHUNK == bhi:
                            scalar.wait_ge(s_cp2, c + 1)
                            dst = out_ext[2 * p + 1 : 2 * p + 2].rearrange(
                                "b c h w -> (b c) h w"
                            )
                            scalar.dma_start(
                                out=dst[:, blo:bhi, :], in_=obs[p][1][bi][:]
                            ).then_inc(s_out, 16)

            @block.tensor
            def _(tensor: bass.BassEngine):
                # HAM warm-up: junk matmuls on not-yet-loaded SBUF while the
                # first input DMAs land; keeps the PE at 8/8 clock for the
                # real stream. banks[3] is first reused by chunk 3 (start=True
                # clears it), well after these complete.
                for wi in range(40):
                    h = wi % 2
                    c0 = h * C_IN
                    tensor.matmul(
                        out=banks[3][h][:],
                        lhsT=wt[c0 : c0 + C_IN, 0, :],
                        rhs=xps[0][c0 : c0 + C_IN, 0:RCHUNK, 0:H],
                        start=True,
                        stop=True,
                    )
                tensor.wait_ge(s_w, 16)
                for p in range(2):
                    for ci in range(NCHUNK):
                        c = p * NCHUNK + ci
                        h0 = ci * RCHUNK
                        if p == 0:
                            if ci == 0:
                                tensor.wait_ge(s_xa, 16)  # rows [0,11)
                            elif ci == 1:
                                tensor.wait_ge(s_x[0], 16)  # rows [11,34)
                            elif ci == 4:
                                tensor.wait_ge(s_x[0], 32)  # rows [34,58)
                        else:
                            if ci == 0:
                                tensor.wait_ge(s_x[1], 16)
                            elif ci == 1:
                                tensor.wait_ge(s_x[1], 32)
                            elif ci == 4:
                                tensor.wait_ge(s_x[1], 48)
                        if c >= 4:
                            # WAR: bank slot c%4 last used by chunk c-4
                            tensor.wait_ge(s_cp, c - 3)
                            tensor.wait_ge(s_cp2, c - 3)
                        for k in range(KS * KS):
                            di, dj = divmod(k, KS)
                            last = k == KS * KS - 1
                            for half in range(2):
                                c0 = half * C_IN
                                mm = tensor.matmul(
                                    out=banks[c % 4][half][:],
                                    lhsT=wt[c0 : c0 + C_IN, k, :],
                                    rhs=xps[p][
                                        c0 : c0 + C_IN,
                                        h0 + di : h0 + di + RCHUNK,
                                        dj : dj + H,
                                    ],
                                    start=(k == 0),
                                    stop=last,
                                )
                                if last and half == 1:
                                    mm.then_inc(s_mm, 2)

            @block.vector
            def _(vector: bass.BassEngine):
                for p in range(2):
                    for ci in range(NCHUNK):
                        c = p * NCHUNK + ci
                        h0 = ci * RCHUNK
                        blo, bhi = next(b for b in OBLOCKS if b[0] <= h0 < b[1])
                        bi = OBLOCKS.index((blo, bhi))
                        vector.wait_ge(s_mm, 2 * (c + 1))
                        vector.tensor_copy(
                            out=obs[p][0][bi][:, h0 - blo : h0 - blo + RCHUNK, :],
                            in_=banks[c % 4][0][:],
                        ).then_inc(s_cp, 1)

    return nc


def _prep_inputs(x, K, mm_dt=MM_DT):
    np_dt = mybir.dt.np(mm_dt)
    x = np.ascontiguousarray(np.asarray(x, dtype=np.float32))
    K = np.ascontiguousarray(np.asarray(K, dtype=np.float32))
    xpad = np.pad(x, ((0, 0), (0, 0), (1, 1), (1, 1))).astype(np_dt)
    Wt = K.reshape(KS * KS * C_IN, C_OUT).reshape(C_IN, KS * KS, C_OUT)
    Wrep = np.ascontiguousarray(np.concatenate([Wt, Wt], axis=0)).astype(np_dt)
    shards = xpad.reshape(N_CORES, BPC, C_IN, HP, HP)
    return [{"x": np.ascontiguousarray(shards[i]), "w": Wrep} for i in range(N_CORES)]


def run(x, K, trace=False, mm_dt=MM_DT):
    nc = build_nc(mm_dt)
    in_maps = _prep_inputs(x, K, mm_dt)
    res = run_bass_kernel_spmd(nc, in_maps, list(range(N_CORES)), trace=trace)
    out = np.concatenate([res.results[i]["out"] for i in range(N_CORES)], axis=0)
    return out, res


def kernel(x, K):
    out, _ = run(x, K, trace=False)
    return out
